# revision 3
# baseline (speedup 1.0000x reference)
"""Bahdanau attention with coverage — Trainium2 Bass kernel.

Full-input contract: kernel(**inputs) takes the unsharded numpy inputs,
shards batch B=64 across 8 NeuronCores (8 batch elements per core),
runs one SPMD Bass kernel, and reassembles the full outputs.

Math per batch element i (S=1024 source positions, U=H=D=512):
    dec_attn = dec_state[i] @ W_dec + b_dec                  # [U]
    z[s,u]   = enc_attn[i,s,u] + coverage[i,s]*W_cov[u] + dec_attn[u]
    scores_s = sum_u tanh(z[s,u]) * w_attn[u]                # [S]
    attn     = softmax(scores)                               # [S]
    c        = sum_s attn_s * enc_output[i,s,:]              # [H]

Device mapping (per core, per batch element b of 8):
  * enc_attn is pre-transposed on host to [U, S] so U sits on SBUF
    partitions.  Then per u-chunk (128 partitions):
      - DVE tensor_scalar: zc = cov_bcast * W_cov[p] + dec_attn[p]
        (coverage row replicated across partitions once per b via GPSIMD)
      - DVE tensor_tensor: z = zc + encT_tile
      - ACT: t = tanh(z)
      - PE:  scores[1,S] += w_chunk.T @ t     (fp32r, PSUM accumulate)
  * softmax: ACT exp with accum_out (free-dim total), DVE reciprocal,
    DVE tensor_scalar scale.  No max-subtraction: |scores| <= sum|w| ~ 18,
    fp32 exp is exact-safe there and softmax is shift-invariant.
  * context: PE transposes attn row into [128,1] chunks (via identity
    matmul) and accumulates attn.T @ enc_output tiles into PSUM [1,H].
"""

import numpy as np

import concourse.bacc as bacc
import concourse.mybir as mybir
import concourse.tile as tile
from concourse import bass_utils

F32 = mybir.dt.float32
F32R = mybir.dt.float32r
ALU = mybir.AluOpType
ACTF = mybir.ActivationFunctionType

B, S, U, H, D = 64, 1024, 512, 512, 512
NCORES = 8
BL = B // NCORES          # batch per core
UC = U // 128             # u chunks per core
SC = S // 128             # s chunks
DC = D // 128             # d chunks

# u-chunks whose z-add runs on GpSimd instead of DVE
GP_ADD_UC = {2, 3}

# test.py reads these for profiling
LAST_RESULTS = None


def _build_body(tc, aps):
    nc = tc.nc
    encT = aps["encT"]      # [BL, U, S]
    enc = aps["enc"]        # [BL, S, H]
    cov = aps["cov"]        # [BL, S]
    decT = aps["decT"]      # [D, BL]
    wdec = aps["wdec"]      # [D, U]
    bdec = aps["bdec"]      # [128, UC]
    wcov = aps["wcov"]      # [128, UC]
    watt = aps["watt"]      # [128, UC]
    attn = aps["attn"]      # [BL, S]  out
    cvec = aps["cvec"]      # [BL, H]  out

    with (
        tc.tile_pool(name="const", bufs=1) as cpool,
        tc.tile_pool(name="encs", bufs=2) as encpool,
        tc.tile_pool(name="work", bufs=3) as wpool,
        tc.tile_pool(name="small", bufs=2) as spool,
        tc.tile_pool(name="psc", bufs=2, space="PSUM") as psc_pool,
        tc.tile_pool(name="pmisc", bufs=2, space="PSUM") as pmisc_pool,
        tc.tile_pool(name="pc", bufs=2, space="PSUM") as pc_pool,
    ):
        # ---- constants / setup ----
        wdec_sb = cpool.tile([128, DC, U], F32)
        nc.sync.dma_start(wdec_sb[:], wdec.rearrange("(c p) u -> p c u", p=128))
        decT_sb = cpool.tile([128, DC, BL], F32)
        nc.sync.dma_start(decT_sb[:], decT.rearrange("(c p) b -> p c b", p=128))
        bdec_sb = cpool.tile([128, UC], F32)
        nc.sync.dma_start(bdec_sb[:], bdec[:])
        wcov_sb = cpool.tile([128, UC], F32)
        nc.sync.dma_start(wcov_sb[:], wcov[:])
        watt_sb = cpool.tile([128, UC], F32R)
        nc.sync.dma_start(watt_sb[:], watt[:])

        ident = cpool.tile([1, 1], F32)
        nc.vector.memset(ident[:], 1.0)

        cov_all_row = cpool.tile([1, BL * S], F32)
        nc.sync.dma_start(cov_all_row[:], cov.rearrange("b s -> (b s)").unsqueeze(0))
        cov_all_rep = cpool.tile([128, BL * S], F32)
        nc.gpsimd.partition_broadcast(cov_all_rep[:], cov_all_row[:])
        cov_reps = [cov_all_rep[:, b * S : (b + 1) * S] for b in range(BL)]

        # dec_attnT[u, b] = sum_d W_dec[d, u] * dec_state[b, d]   (+ b_dec)
        psum_dec = pmisc_pool.tile([128, UC * BL], F32, tag="misc", name="psum_dec")
        for uc in range(UC):
            for dc in range(DC):
                nc.tensor.matmul(
                    psum_dec[:, uc * BL : (uc + 1) * BL],
                    lhsT=wdec_sb[:, dc, uc * 128 : (uc + 1) * 128],
                    rhs=decT_sb[:, dc, :],
                    start=(dc == 0),
                    stop=(dc == DC - 1),
                )
        dec_sb = cpool.tile([128, UC * BL], F32)
        for uc in range(UC):
            nc.vector.tensor_scalar(
                dec_sb[:, uc * BL : (uc + 1) * BL],
                in0=psum_dec[:, uc * BL : (uc + 1) * BL],
                scalar1=bdec_sb[:, uc : uc + 1],
                scalar2=None,
                op0=ALU.add,
            )

        # ---- main loop over local batch ----
        for b in range(BL):
            # phase A: scores
            encT_t = encpool.tile([128, UC, S], F32, tag="encT", name="encT_t")
            nc.sync.dma_start(encT_t[:], encT[b].rearrange("(c p) s -> p c s", p=128))
            psum_sc = psc_pool.tile([1, S], F32, name="psum_sc")
            for uc in range(UC):
                z = wpool.tile([128, S], F32, tag="z", name="z")
                nc.vector.tensor_scalar(
                    z[:],
                    in0=cov_reps[b],
                    scalar1=wcov_sb[:, uc : uc + 1],
                    scalar2=None,
                    op0=ALU.mult,
                )
                add_eng = nc.gpsimd if uc in GP_ADD_UC else nc.vector
                add_eng.tensor_tensor(z[:], z[:], encT_t[:, uc, :], op=ALU.add)
                t = wpool.tile([128, S], F32R, tag="t", name="t")
                nc.scalar.activation(
                    t[:], z[:], ACTF.Tanh,
                    bias=dec_sb[:, uc * BL + b : uc * BL + b + 1],
                )
                for sl in range(S // 512):
                    nc.tensor.matmul(
                        psum_sc[0:1, sl * 512 : (sl + 1) * 512],
                        lhsT=watt_sb[:, uc : uc + 1],
                        rhs=t[:, sl * 512 : (sl + 1) * 512],
                        start=(uc == 0),
                        stop=(uc == UC - 1),
                        skip_group_check=True,
                    )

            # phase B: softmax (no max subtraction; scores are bounded)
            escore = spool.tile([1, S], F32, tag="escore", name="escore")
            total = spool.tile([1, 1], F32, tag="total", name="total")
            nc.scalar.activation(
                escore[:], psum_sc[0:1, :], ACTF.Exp, accum_out=total[:]
            )
            recip = spool.tile([1, 1], F32, tag="recip", name="recip")
            nc.vector.reciprocal(recip[:], total[:])
            attn_row = spool.tile([1, S], F32, tag="attnrow", name="attn_row")
            nc.vector.tensor_scalar(
                attn_row[:], in0=escore[:], scalar1=recip[:], scalar2=None,
                op0=ALU.mult,
            )
            nc.sync.dma_start(attn[b : b + 1, :], attn_row[:])

            # transpose attn row into [128, SC] (s on partitions)
            psum_T = pmisc_pool.tile([128, SC], F32, tag="misc", name="psum_T")
            for c in range(SC):
                nc.tensor.transpose(
                    psum_T[:, c : c + 1],
                    attn_row[0:1, c * 128 : (c + 1) * 128],
                    ident[:],
                )
            attn_T = spool.tile([128, SC], F32R, tag="attnT", name="attn_T")
            nc.vector.tensor_copy(attn_T[:], psum_T[:])

            # phase C: context vector
            enc_t = encpool.tile([128, SC, H], F32R, tag="enc", name="enc_t")
            nc.sync.dma_start(enc_t[:], enc[b].rearrange("(c p) h -> p c h", p=128))
            psum_c = pc_pool.tile([1, H], F32, name="psum_c")
            for c in range(SC):
                nc.tensor.matmul(
                    psum_c[0:1, :],
                    lhsT=attn_T[:, c : c + 1],
                    rhs=enc_t[:, c, :],
                    start=(c == 0),
                    stop=(c == SC - 1),
                )
            cvec_row = spool.tile([1, H], F32, tag="cvecrow", name="cvec_row")
            nc.scalar.copy(cvec_row[:], psum_c[0:1, :])
            nc.sync.dma_start(cvec[b : b + 1, :], cvec_row[:])


_NC = None


def build_nc():
    global _NC
    if _NC is not None:
        return _NC
    nc = bacc.Bacc("TRN2", target_bir_lowering=False, debug=False)
    aps = {}
    for name, shape in [
        ("encT", [BL, U, S]),
        ("cov", [BL, S]),
        ("decT", [D, BL]),
        ("wdec", [D, U]),
        ("bdec", [128, UC]),
        ("wcov", [128, UC]),
    ]:
        aps[name] = nc.dram_tensor(name, shape, F32, kind="ExternalInput").ap()
    for name, shape in [("enc", [BL, S, H]), ("watt", [128, UC])]:
        aps[name] = nc.dram_tensor(name, shape, F32R, kind="ExternalInput").ap()
    for name, shape in [("attn", [BL, S]), ("cvec", [BL, H])]:
        aps[name] = nc.dram_tensor(name, shape, F32, kind="ExternalOutput").ap()

    with tile.TileContext(nc) as tc:
        _build_body(tc, aps)
    nc.compile()
    _NC = nc
    return nc


def make_in_maps(enc_output, enc_attn, coverage_vector, dec_state, W_dec, b_dec,
                 W_cov, w_attn):
    enc_output = np.asarray(enc_output, dtype=np.float32)
    enc_attn = np.asarray(enc_attn, dtype=np.float32)
    coverage_vector = np.asarray(coverage_vector, dtype=np.float32)
    dec_state = np.asarray(dec_state, dtype=np.float32)
    wdec = np.ascontiguousarray(np.asarray(W_dec, dtype=np.float32))
    bdec = np.ascontiguousarray(np.asarray(b_dec, dtype=np.float32).reshape(UC, 128).T)
    wcov = np.ascontiguousarray(
        np.asarray(W_cov, dtype=np.float32)[0].reshape(UC, 128).T
    )
    watt = np.ascontiguousarray(
        np.asarray(w_attn, dtype=np.float32)[:, 0].reshape(UC, 128).T
    )
    in_maps = []
    for i in range(NCORES):
        sl = slice(i * BL, (i + 1) * BL)
        in_maps.append({
            "encT": np.ascontiguousarray(enc_attn[sl].transpose(0, 2, 1)),
            "enc": np.ascontiguousarray(enc_output[sl]),
            "cov": np.ascontiguousarray(coverage_vector[sl]),
            "decT": np.ascontiguousarray(dec_state[sl].T),
            "wdec": wdec,
            "bdec": bdec,
            "wcov": wcov,
            "watt": watt,
        })
    return in_maps


def kernel(enc_output, enc_attn, coverage_vector, dec_state, W_dec, b_dec,
           W_cov, w_attn, **run_kwargs):
    global LAST_RESULTS
    nc = build_nc()
    in_maps = make_in_maps(enc_output, enc_attn, coverage_vector, dec_state,
                           W_dec, b_dec, W_cov, w_attn)
    res = bass_utils.run_bass_kernel_spmd(
        nc, in_maps, core_ids=list(range(NCORES)), **run_kwargs
    )
    LAST_RESULTS = res
    attn = np.concatenate([r["attn"] for r in res.results], axis=0)
    cvec = np.concatenate([r["cvec"] for r in res.results], axis=0)
    return attn.reshape(B, S, 1), cvec


# revision 5
# speedup vs baseline: 1.1022x; 1.1022x over previous
"""Bahdanau attention with coverage — Trainium2 Bass kernel.

Full-input contract: kernel(**inputs) takes the unsharded numpy inputs,
shards batch B=64 across 8 NeuronCores (8 batch elements per core),
runs one SPMD Bass kernel, and reassembles the full outputs.

Math per batch element i (S=1024 source positions, U=H=D=512):
    dec_attn = dec_state[i] @ W_dec + b_dec                  # [U]
    z[s,u]   = enc_attn[i,s,u] + coverage[i,s]*W_cov[u] + dec_attn[u]
    scores_s = sum_u tanh(z[s,u]) * w_attn[u]                # [S]
    attn     = softmax(scores)                               # [S]
    c        = sum_s attn_s * enc_output[i,s,:]              # [H]

Device mapping (per core, per batch element b of 8):
  * enc_attn is pre-transposed on host to [U, S] so U sits on SBUF
    partitions.  Then per u-chunk (128 partitions):
      - DVE tensor_scalar: zc = cov_bcast * W_cov[p] + dec_attn[p]
        (coverage row replicated across partitions once per b via GPSIMD)
      - DVE tensor_tensor: z = zc + encT_tile
      - ACT: t = tanh(z)
      - PE:  scores[1,S] += w_chunk.T @ t     (fp32r, PSUM accumulate)
  * softmax: ACT exp with accum_out (free-dim total), DVE reciprocal,
    DVE tensor_scalar scale.  No max-subtraction: |scores| <= sum|w| ~ 18,
    fp32 exp is exact-safe there and softmax is shift-invariant.
  * context: PE transposes attn row into [128,1] chunks (via identity
    matmul) and accumulates attn.T @ enc_output tiles into PSUM [1,H].
"""

import ml_dtypes
import numpy as np

import concourse.bacc as bacc
import concourse.mybir as mybir
import concourse.tile as tile
from concourse import bass_utils

F32 = mybir.dt.float32
F32R = mybir.dt.float32r
BF16 = mybir.dt.bfloat16
ALU = mybir.AluOpType
ACTF = mybir.ActivationFunctionType

B, S, U, H, D = 64, 1024, 512, 512, 512
NCORES = 8
BL = B // NCORES          # batch per core
UC = U // 128             # u chunks per core
SC = S // 128             # s chunks
DC = D // 128             # d chunks

# u-chunks whose z-add runs on GpSimd instead of DVE
GP_ADD_UC = set()

# test.py reads these for profiling
LAST_RESULTS = None


def _build_body(tc, aps):
    nc = tc.nc
    encT = aps["encT"]      # [BL, U, S]
    enc = aps["enc"]        # [BL, S, H]
    cov = aps["cov"]        # [BL, S]
    decT = aps["decT"]      # [D, BL]
    wdec = aps["wdec"]      # [D, U]
    bdec = aps["bdec"]      # [128, UC]
    wcov = aps["wcov"]      # [128, UC]
    watt = aps["watt"]      # [128, UC]
    attn = aps["attn"]      # [BL, S]  out
    cvec = aps["cvec"]      # [BL, H]  out

    with (
        tc.tile_pool(name="const", bufs=1) as cpool,
        tc.tile_pool(name="encs", bufs=3) as encpool,
        tc.tile_pool(name="work", bufs=4) as wpool,
        tc.tile_pool(name="small", bufs=2) as spool,
        tc.tile_pool(name="psc", bufs=2, space="PSUM") as psc_pool,
        tc.tile_pool(name="pmisc", bufs=2, space="PSUM") as pmisc_pool,
        tc.tile_pool(name="pc", bufs=2, space="PSUM") as pc_pool,
    ):
        # ---- constants / setup ----
        wdec_sb = cpool.tile([128, DC, U], F32)
        nc.sync.dma_start(wdec_sb[:], wdec.rearrange("(c p) u -> p c u", p=128))
        decT_sb = cpool.tile([128, DC, BL], F32)
        nc.sync.dma_start(decT_sb[:], decT.rearrange("(c p) b -> p c b", p=128))
        bdec_sb = cpool.tile([128, UC], F32)
        nc.sync.dma_start(bdec_sb[:], bdec[:])
        wcov_sb = cpool.tile([128, UC], F32)
        nc.sync.dma_start(wcov_sb[:], wcov[:])
        watt_sb = cpool.tile([128, UC], F32R)
        nc.sync.dma_start(watt_sb[:], watt[:])

        ident = cpool.tile([1, 1], F32)
        nc.vector.memset(ident[:], 1.0)

        cov_all_row = cpool.tile([1, BL * S], BF16)
        nc.sync.dma_start(cov_all_row[:], cov.rearrange("b s -> (b s)").unsqueeze(0))
        cov_all_rep = cpool.tile([128, BL * S], BF16)
        for b in range(BL):
            nc.gpsimd.partition_broadcast(
                cov_all_rep[:, b * S : (b + 1) * S],
                cov_all_row[:, b * S : (b + 1) * S],
            )
        cov_reps = [cov_all_rep[:, b * S : (b + 1) * S] for b in range(BL)]

        # dec_attnT[u, b] = sum_d W_dec[d, u] * dec_state[b, d]   (+ b_dec)
        psum_dec = pmisc_pool.tile([128, UC * BL], F32, tag="misc", name="psum_dec")
        for uc in range(UC):
            for dc in range(DC):
                nc.tensor.matmul(
                    psum_dec[:, uc * BL : (uc + 1) * BL],
                    lhsT=wdec_sb[:, dc, uc * 128 : (uc + 1) * 128],
                    rhs=decT_sb[:, dc, :],
                    start=(dc == 0),
                    stop=(dc == DC - 1),
                )
        dec_sb = cpool.tile([128, UC * BL], F32)
        for uc in range(UC):
            nc.vector.tensor_scalar(
                dec_sb[:, uc * BL : (uc + 1) * BL],
                in0=psum_dec[:, uc * BL : (uc + 1) * BL],
                scalar1=bdec_sb[:, uc : uc + 1],
                scalar2=None,
                op0=ALU.add,
            )

        # ---- main loop over local batch ----
        for b in range(BL):
            # phase A: scores
            encT_t = encpool.tile([128, UC, S], F32, tag="encT", name="encT_t")
            nc.sync.dma_start(encT_t[:], encT[b].rearrange("(c p) s -> p c s", p=128))
            psum_sc = psc_pool.tile([1, S], F32, name="psum_sc")
            for uc in range(UC):
                covw = wpool.tile([128, S], BF16, tag="covw", name="covw")
                nc.vector.tensor_scalar(
                    covw[:],
                    in0=cov_reps[b],
                    scalar1=wcov_sb[:, uc : uc + 1],
                    scalar2=None,
                    op0=ALU.mult,
                )
                z = wpool.tile([128, S], F32, tag="z", name="z")
                add_eng = nc.gpsimd if uc in GP_ADD_UC else nc.vector
                add_eng.tensor_tensor(z[:], covw[:], encT_t[:, uc, :], op=ALU.add)
                t = wpool.tile([128, S], F32R, tag="t", name="t")
                nc.scalar.activation(
                    t[:], z[:], ACTF.Tanh,
                    bias=dec_sb[:, uc * BL + b : uc * BL + b + 1],
                )
                for sl in range(S // 512):
                    nc.tensor.matmul(
                        psum_sc[0:1, sl * 512 : (sl + 1) * 512],
                        lhsT=watt_sb[:, uc : uc + 1],
                        rhs=t[:, sl * 512 : (sl + 1) * 512],
                        start=(uc == 0),
                        stop=(uc == UC - 1),
                        skip_group_check=True,
                    )

            # phase B: softmax (no max subtraction; scores are bounded)
            escore = spool.tile([1, S], F32, tag="escore", name="escore")
            total = spool.tile([1, 1], F32, tag="total", name="total")
            nc.scalar.activation(
                escore[:], psum_sc[0:1, :], ACTF.Exp, accum_out=total[:]
            )
            recip = spool.tile([1, 1], F32, tag="recip", name="recip")
            nc.vector.reciprocal(recip[:], total[:])
            attn_row = spool.tile([1, S], F32, tag="attnrow", name="attn_row")
            nc.vector.tensor_scalar(
                attn_row[:], in0=escore[:], scalar1=recip[:], scalar2=None,
                op0=ALU.mult,
            )
            nc.sync.dma_start(attn[b : b + 1, :], attn_row[:])

            # transpose attn row into [128, SC] (s on partitions)
            psum_T = pmisc_pool.tile([128, SC], F32, tag="misc", name="psum_T")
            for c in range(SC):
                nc.tensor.transpose(
                    psum_T[:, c : c + 1],
                    attn_row[0:1, c * 128 : (c + 1) * 128],
                    ident[:],
                )
            attn_T = spool.tile([128, SC], F32R, tag="attnT", name="attn_T")
            nc.vector.tensor_copy(attn_T[:], psum_T[:])

            # phase C: context vector
            enc_t = encpool.tile([128, SC, H], F32R, tag="enc", name="enc_t")
            nc.sync.dma_start(enc_t[:], enc[b].rearrange("(c p) h -> p c h", p=128))
            psum_c = pc_pool.tile([1, H], F32, name="psum_c")
            for c in range(SC):
                nc.tensor.matmul(
                    psum_c[0:1, :],
                    lhsT=attn_T[:, c : c + 1],
                    rhs=enc_t[:, c, :],
                    start=(c == 0),
                    stop=(c == SC - 1),
                )
            cvec_row = spool.tile([1, H], F32, tag="cvecrow", name="cvec_row")
            nc.scalar.copy(cvec_row[:], psum_c[0:1, :])
            nc.sync.dma_start(cvec[b : b + 1, :], cvec_row[:])


_NC = None


def build_nc():
    global _NC
    if _NC is not None:
        return _NC
    nc = bacc.Bacc("TRN2", target_bir_lowering=False, debug=False)
    aps = {}
    for name, shape in [
        ("encT", [BL, U, S]),
        ("decT", [D, BL]),
        ("wdec", [D, U]),
        ("bdec", [128, UC]),
        ("wcov", [128, UC]),
    ]:
        aps[name] = nc.dram_tensor(name, shape, F32, kind="ExternalInput").ap()
    aps["cov"] = nc.dram_tensor("cov", [BL, S], BF16, kind="ExternalInput").ap()
    for name, shape in [("enc", [BL, S, H]), ("watt", [128, UC])]:
        aps[name] = nc.dram_tensor(name, shape, F32R, kind="ExternalInput").ap()
    for name, shape in [("attn", [BL, S]), ("cvec", [BL, H])]:
        aps[name] = nc.dram_tensor(name, shape, F32, kind="ExternalOutput").ap()

    with tile.TileContext(nc) as tc:
        _build_body(tc, aps)
    nc.compile()
    _NC = nc
    return nc


def make_in_maps(enc_output, enc_attn, coverage_vector, dec_state, W_dec, b_dec,
                 W_cov, w_attn):
    enc_output = np.asarray(enc_output, dtype=np.float32)
    enc_attn = np.asarray(enc_attn, dtype=np.float32)
    coverage_vector = np.asarray(coverage_vector, dtype=np.float32).astype(
        ml_dtypes.bfloat16
    )
    dec_state = np.asarray(dec_state, dtype=np.float32)
    wdec = np.ascontiguousarray(np.asarray(W_dec, dtype=np.float32))
    bdec = np.ascontiguousarray(np.asarray(b_dec, dtype=np.float32).reshape(UC, 128).T)
    wcov = np.ascontiguousarray(
        np.asarray(W_cov, dtype=np.float32)[0].reshape(UC, 128).T
    )
    watt = np.ascontiguousarray(
        np.asarray(w_attn, dtype=np.float32)[:, 0].reshape(UC, 128).T
    )
    in_maps = []
    for i in range(NCORES):
        sl = slice(i * BL, (i + 1) * BL)
        in_maps.append({
            "encT": np.ascontiguousarray(enc_attn[sl].transpose(0, 2, 1)),
            "enc": np.ascontiguousarray(enc_output[sl]),
            "cov": np.ascontiguousarray(coverage_vector[sl]),
            "decT": np.ascontiguousarray(dec_state[sl].T),
            "wdec": wdec,
            "bdec": bdec,
            "wcov": wcov,
            "watt": watt,
        })
    return in_maps


def kernel(enc_output, enc_attn, coverage_vector, dec_state, W_dec, b_dec,
           W_cov, w_attn, **run_kwargs):
    global LAST_RESULTS
    nc = build_nc()
    in_maps = make_in_maps(enc_output, enc_attn, coverage_vector, dec_state,
                           W_dec, b_dec, W_cov, w_attn)
    res = bass_utils.run_bass_kernel_spmd(
        nc, in_maps, core_ids=list(range(NCORES)), **run_kwargs
    )
    LAST_RESULTS = res
    attn = np.concatenate([r["attn"] for r in res.results], axis=0)
    cvec = np.concatenate([r["cvec"] for r in res.results], axis=0)
    return attn.reshape(B, S, 1), cvec


# revision 6
# speedup vs baseline: 1.4430x; 1.3091x over previous
"""Bahdanau attention with coverage — Trainium2 Bass kernel.

Full-input contract: kernel(**inputs) takes the unsharded numpy inputs,
shards batch B=64 across 8 NeuronCores (8 batch elements per core),
runs one SPMD Bass kernel, and reassembles the full outputs.

Math per batch element i (S=1024 source positions, U=H=D=512):
    dec_attn = dec_state[i] @ W_dec + b_dec                  # [U]
    z[s,u]   = enc_attn[i,s,u] + coverage[i,s]*W_cov[u] + dec_attn[u]
    scores_s = sum_u tanh(z[s,u]) * w_attn[u]                # [S]
    attn     = softmax(scores)                               # [S]
    c        = sum_s attn_s * enc_output[i,s,:]              # [H]

Device mapping (per core, per batch element b of 8):
  * enc_attn is pre-transposed on host to [U, S] so U sits on SBUF
    partitions.  Then per u-chunk (128 partitions):
      - DVE tensor_scalar: zc = cov_bcast * W_cov[p] + dec_attn[p]
        (coverage row replicated across partitions once per b via GPSIMD)
      - DVE tensor_tensor: z = zc + encT_tile
      - ACT: t = tanh(z)
      - PE:  scores[1,S] += w_chunk.T @ t     (fp32r, PSUM accumulate)
  * softmax: ACT exp with accum_out (free-dim total), DVE reciprocal,
    DVE tensor_scalar scale.  No max-subtraction: |scores| <= sum|w| ~ 18,
    fp32 exp is exact-safe there and softmax is shift-invariant.
  * context: PE transposes attn row into [128,1] chunks (via identity
    matmul) and accumulates attn.T @ enc_output tiles into PSUM [1,H].
"""

import ml_dtypes
import numpy as np

import concourse.bacc as bacc
import concourse.mybir as mybir
import concourse.tile as tile
from concourse import bass_utils

F32 = mybir.dt.float32
F32R = mybir.dt.float32r
BF16 = mybir.dt.bfloat16
ALU = mybir.AluOpType
ACTF = mybir.ActivationFunctionType

B, S, U, H, D = 64, 1024, 512, 512, 512
NCORES = 8
BL = B // NCORES          # batch per core
UC = U // 128             # u chunks per core
SC = S // 128             # s chunks
DC = D // 128             # d chunks

# u-chunks whose z-add runs on GpSimd instead of DVE
GP_ADD_UC = set()

# test.py reads these for profiling
LAST_RESULTS = None


def _build_body(tc, aps):
    nc = tc.nc
    encT = aps["encT"]      # [BL, U, S]
    enc = aps["enc"]        # [BL, S, H]
    cov = aps["cov"]        # [BL, S]
    decT = aps["decT"]      # [D, BL]
    wdec = aps["wdec"]      # [D, U]
    bdec = aps["bdec"]      # [128, UC]
    wcov = aps["wcov"]      # [128, UC]
    watt = aps["watt"]      # [128, UC]
    attn = aps["attn"]      # [BL, S]  out
    cvec = aps["cvec"]      # [BL, H]  out

    with (
        tc.tile_pool(name="const", bufs=1) as cpool,
        tc.tile_pool(name="encs", bufs=3) as encpool,
        tc.tile_pool(name="work", bufs=4) as wpool,
        tc.tile_pool(name="small", bufs=2) as spool,
        tc.tile_pool(name="psc", bufs=2, space="PSUM") as psc_pool,
        tc.tile_pool(name="pmisc", bufs=2, space="PSUM") as pmisc_pool,
        tc.tile_pool(name="pc", bufs=2, space="PSUM") as pc_pool,
    ):
        # ---- constants / setup ----
        wdec_sb = cpool.tile([128, DC, U], F32)
        nc.sync.dma_start(wdec_sb[:], wdec.rearrange("(c p) u -> p c u", p=128))
        decT_sb = cpool.tile([128, DC, BL], F32)
        nc.sync.dma_start(decT_sb[:], decT.rearrange("(c p) b -> p c b", p=128))
        bdec_sb = cpool.tile([128, UC], F32)
        nc.sync.dma_start(bdec_sb[:], bdec[:])
        wcov_sb = cpool.tile([128, UC], F32)
        nc.sync.dma_start(wcov_sb[:], wcov[:])
        watt_sb = cpool.tile([128, UC], BF16)
        nc.sync.dma_start(watt_sb[:], watt[:])

        ident = cpool.tile([1, 1], F32)
        nc.vector.memset(ident[:], 1.0)

        cov_all_row = cpool.tile([1, BL * S], BF16)
        nc.sync.dma_start(cov_all_row[:], cov.rearrange("b s -> (b s)").unsqueeze(0))
        cov_all_rep = cpool.tile([128, BL * S], BF16)
        for b in range(BL):
            nc.gpsimd.partition_broadcast(
                cov_all_rep[:, b * S : (b + 1) * S],
                cov_all_row[:, b * S : (b + 1) * S],
            )
        cov_reps = [cov_all_rep[:, b * S : (b + 1) * S] for b in range(BL)]

        # dec_attnT[u, b] = sum_d W_dec[d, u] * dec_state[b, d]   (+ b_dec)
        psum_dec = pmisc_pool.tile([128, UC * BL], F32, tag="misc", name="psum_dec")
        for uc in range(UC):
            for dc in range(DC):
                nc.tensor.matmul(
                    psum_dec[:, uc * BL : (uc + 1) * BL],
                    lhsT=wdec_sb[:, dc, uc * 128 : (uc + 1) * 128],
                    rhs=decT_sb[:, dc, :],
                    start=(dc == 0),
                    stop=(dc == DC - 1),
                )
        dec_sb = cpool.tile([128, UC * BL], F32)
        for uc in range(UC):
            nc.vector.tensor_scalar(
                dec_sb[:, uc * BL : (uc + 1) * BL],
                in0=psum_dec[:, uc * BL : (uc + 1) * BL],
                scalar1=bdec_sb[:, uc : uc + 1],
                scalar2=None,
                op0=ALU.add,
            )

        # ---- main loop over local batch ----
        for b in range(BL):
            # phase A: scores
            encT_t = encpool.tile([128, UC, S], BF16, tag="encT", name="encT_t")
            nc.sync.dma_start(encT_t[:], encT[b].rearrange("(c p) s -> p c s", p=128))
            psum_sc = psc_pool.tile([1, S], F32, name="psum_sc")
            for uc in range(UC):
                covw = wpool.tile([128, S], BF16, tag="covw", name="covw")
                nc.vector.tensor_scalar(
                    covw[:],
                    in0=cov_reps[b],
                    scalar1=wcov_sb[:, uc : uc + 1],
                    scalar2=None,
                    op0=ALU.mult,
                )
                z = wpool.tile([128, S], BF16, tag="z", name="z")
                add_eng = nc.gpsimd if uc in GP_ADD_UC else nc.vector
                add_eng.tensor_tensor(z[:], covw[:], encT_t[:, uc, :], op=ALU.add)
                t = wpool.tile([128, S], BF16, tag="t", name="t")
                nc.scalar.activation(
                    t[:], z[:], ACTF.Tanh,
                    bias=dec_sb[:, uc * BL + b : uc * BL + b + 1],
                )
                for sl in range(S // 512):
                    nc.tensor.matmul(
                        psum_sc[0:1, sl * 512 : (sl + 1) * 512],
                        lhsT=watt_sb[:, uc : uc + 1],
                        rhs=t[:, sl * 512 : (sl + 1) * 512],
                        start=(uc == 0),
                        stop=(uc == UC - 1),
                        skip_group_check=True,
                    )

            # phase B: softmax (no max subtraction; scores are bounded)
            escore = spool.tile([1, S], F32, tag="escore", name="escore")
            total = spool.tile([1, 1], F32, tag="total", name="total")
            nc.scalar.activation(
                escore[:], psum_sc[0:1, :], ACTF.Exp, accum_out=total[:]
            )
            recip = spool.tile([1, 1], F32, tag="recip", name="recip")
            nc.vector.reciprocal(recip[:], total[:])
            attn_row = spool.tile([1, S], F32, tag="attnrow", name="attn_row")
            nc.vector.tensor_scalar(
                attn_row[:], in0=escore[:], scalar1=recip[:], scalar2=None,
                op0=ALU.mult,
            )
            nc.sync.dma_start(attn[b : b + 1, :], attn_row[:])

            # transpose attn row into [128, SC] (s on partitions)
            psum_T = pmisc_pool.tile([128, SC], F32, tag="misc", name="psum_T")
            for c in range(SC):
                nc.tensor.transpose(
                    psum_T[:, c : c + 1],
                    attn_row[0:1, c * 128 : (c + 1) * 128],
                    ident[:],
                )
            attn_T = spool.tile([128, SC], BF16, tag="attnT", name="attn_T")
            nc.vector.tensor_copy(attn_T[:], psum_T[:])

            # phase C: context vector
            enc_t = encpool.tile([128, SC, H], BF16, tag="enc", name="enc_t")
            nc.sync.dma_start(enc_t[:], enc[b].rearrange("(c p) h -> p c h", p=128))
            psum_c = pc_pool.tile([1, H], F32, name="psum_c")
            for c in range(SC):
                nc.tensor.matmul(
                    psum_c[0:1, :],
                    lhsT=attn_T[:, c : c + 1],
                    rhs=enc_t[:, c, :],
                    start=(c == 0),
                    stop=(c == SC - 1),
                )
            cvec_row = spool.tile([1, H], F32, tag="cvecrow", name="cvec_row")
            nc.scalar.copy(cvec_row[:], psum_c[0:1, :])
            nc.sync.dma_start(cvec[b : b + 1, :], cvec_row[:])


_NC = None


def build_nc():
    global _NC
    if _NC is not None:
        return _NC
    nc = bacc.Bacc("TRN2", target_bir_lowering=False, debug=False)
    aps = {}
    for name, shape in [
        ("decT", [D, BL]),
        ("wdec", [D, U]),
        ("bdec", [128, UC]),
        ("wcov", [128, UC]),
    ]:
        aps[name] = nc.dram_tensor(name, shape, F32, kind="ExternalInput").ap()
    for name, shape in [
        ("cov", [BL, S]),
        ("enc", [BL, S, H]),
        ("encT", [BL, U, S]),
        ("watt", [128, UC]),
    ]:
        aps[name] = nc.dram_tensor(name, shape, BF16, kind="ExternalInput").ap()
    for name, shape in [("attn", [BL, S]), ("cvec", [BL, H])]:
        aps[name] = nc.dram_tensor(name, shape, F32, kind="ExternalOutput").ap()

    with tile.TileContext(nc) as tc:
        _build_body(tc, aps)
    nc.compile()
    _NC = nc
    return nc


def make_in_maps(enc_output, enc_attn, coverage_vector, dec_state, W_dec, b_dec,
                 W_cov, w_attn):
    enc_output = np.asarray(enc_output, dtype=np.float32)
    enc_attn = np.asarray(enc_attn, dtype=np.float32)
    coverage_vector = np.asarray(coverage_vector, dtype=np.float32).astype(
        ml_dtypes.bfloat16
    )
    dec_state = np.asarray(dec_state, dtype=np.float32)
    wdec = np.ascontiguousarray(np.asarray(W_dec, dtype=np.float32))
    bdec = np.ascontiguousarray(np.asarray(b_dec, dtype=np.float32).reshape(UC, 128).T)
    wcov = np.ascontiguousarray(
        np.asarray(W_cov, dtype=np.float32)[0].reshape(UC, 128).T
    )
    watt = np.ascontiguousarray(
        np.asarray(w_attn, dtype=np.float32)[:, 0].reshape(UC, 128).T
    ).astype(ml_dtypes.bfloat16)
    in_maps = []
    for i in range(NCORES):
        sl = slice(i * BL, (i + 1) * BL)
        in_maps.append({
            "encT": np.ascontiguousarray(
                enc_attn[sl].transpose(0, 2, 1)
            ).astype(ml_dtypes.bfloat16),
            "enc": np.ascontiguousarray(enc_output[sl]).astype(ml_dtypes.bfloat16),
            "cov": np.ascontiguousarray(coverage_vector[sl]),
            "decT": np.ascontiguousarray(dec_state[sl].T),
            "wdec": wdec,
            "bdec": bdec,
            "wcov": wcov,
            "watt": watt,
        })
    return in_maps


def kernel(enc_output, enc_attn, coverage_vector, dec_state, W_dec, b_dec,
           W_cov, w_attn, **run_kwargs):
    global LAST_RESULTS
    nc = build_nc()
    in_maps = make_in_maps(enc_output, enc_attn, coverage_vector, dec_state,
                           W_dec, b_dec, W_cov, w_attn)
    res = bass_utils.run_bass_kernel_spmd(
        nc, in_maps, core_ids=list(range(NCORES)), **run_kwargs
    )
    LAST_RESULTS = res
    attn = np.concatenate([r["attn"] for r in res.results], axis=0)
    cvec = np.concatenate([r["cvec"] for r in res.results], axis=0)
    return attn.reshape(B, S, 1), cvec


# revision 7
# speedup vs baseline: 1.4971x; 1.0375x over previous
"""Bahdanau attention with coverage — Trainium2 Bass kernel.

Full-input contract: kernel(**inputs) takes the unsharded numpy inputs,
shards batch B=64 across 8 NeuronCores (8 batch elements per core),
runs one SPMD Bass kernel, and reassembles the full outputs.

Math per batch element i (S=1024 source positions, U=H=D=512):
    dec_attn = dec_state[i] @ W_dec + b_dec                  # [U]
    z[s,u]   = enc_attn[i,s,u] + coverage[i,s]*W_cov[u] + dec_attn[u]
    scores_s = sum_u tanh(z[s,u]) * w_attn[u]                # [S]
    attn     = softmax(scores)                               # [S]
    c        = sum_s attn_s * enc_output[i,s,:]              # [H]

Device mapping (per core, per batch element b of 8):
  * enc_attn is pre-transposed on host to [U, S] so U sits on SBUF
    partitions.  Then per u-chunk (128 partitions):
      - DVE tensor_scalar: zc = cov_bcast * W_cov[p] + dec_attn[p]
        (coverage row replicated across partitions once per b via GPSIMD)
      - DVE tensor_tensor: z = zc + encT_tile
      - ACT: t = tanh(z)
      - PE:  scores[1,S] += w_chunk.T @ t     (fp32r, PSUM accumulate)
  * softmax: ACT exp with accum_out (free-dim total), DVE reciprocal,
    DVE tensor_scalar scale.  No max-subtraction: |scores| <= sum|w| ~ 18,
    fp32 exp is exact-safe there and softmax is shift-invariant.
  * context: PE transposes attn row into [128,1] chunks (via identity
    matmul) and accumulates attn.T @ enc_output tiles into PSUM [1,H].
"""

import ml_dtypes
import numpy as np

import concourse.bacc as bacc
import concourse.mybir as mybir
import concourse.tile as tile
from concourse import bass_utils

F32 = mybir.dt.float32
F32R = mybir.dt.float32r
BF16 = mybir.dt.bfloat16
ALU = mybir.AluOpType
ACTF = mybir.ActivationFunctionType

B, S, U, H, D = 64, 1024, 512, 512, 512
NCORES = 8
BL = B // NCORES          # batch per core
UC = U // 128             # u chunks per core
SC = S // 128             # s chunks
DC = D // 128             # d chunks

# u-chunks whose z-add runs on GpSimd instead of DVE
GP_ADD_UC = set()

# PE HAM warmer: dummy matmuls interleaved to keep the clock at 2.4 GHz
WARM_PER_UC = 2
WARM_PER_B = 4

# test.py reads these for profiling
LAST_RESULTS = None


def _build_body(tc, aps):
    nc = tc.nc
    encT = aps["encT"]      # [BL, U, S]
    enc = aps["enc"]        # [BL, S, H]
    cov = aps["cov"]        # [BL, S]
    decT = aps["decT"]      # [D, BL]
    wdec = aps["wdec"]      # [D, U]
    bdec = aps["bdec"]      # [128, UC]
    wcov = aps["wcov"]      # [128, UC]
    watt = aps["watt"]      # [128, UC]
    attn = aps["attn"]      # [BL, S]  out
    cvec = aps["cvec"]      # [BL, H]  out

    with (
        tc.tile_pool(name="const", bufs=1) as cpool,
        tc.tile_pool(name="encs", bufs=3) as encpool,
        tc.tile_pool(name="work", bufs=4) as wpool,
        tc.tile_pool(name="small", bufs=2) as spool,
        tc.tile_pool(name="psc", bufs=2, space="PSUM") as psc_pool,
        tc.tile_pool(name="pmisc", bufs=1, space="PSUM") as pmisc_pool,
        tc.tile_pool(name="pc", bufs=2, space="PSUM") as pc_pool,
        tc.tile_pool(name="pwarm", bufs=1, space="PSUM") as pwarm_pool,
    ):
        # ---- constants / setup ----
        # coverage first: the broadcast gates the first z chain
        cov_all_row = cpool.tile([1, BL * S], BF16)
        nc.scalar.dma_start(cov_all_row[:], cov.rearrange("b s -> (b s)").unsqueeze(0))
        cov_all_rep = cpool.tile([128, BL * S], BF16)
        HB = BL // 2
        for h in range(2):
            nc.gpsimd.partition_broadcast(
                cov_all_rep[:, h * HB * S : (h + 1) * HB * S],
                cov_all_row[:, h * HB * S : (h + 1) * HB * S],
            )
        cov_reps = [cov_all_rep[:, b * S : (b + 1) * S] for b in range(BL)]

        wdec_sb = cpool.tile([128, DC, U], F32)
        nc.scalar.dma_start(wdec_sb[:], wdec.rearrange("(c p) u -> p c u", p=128))
        decT_sb = cpool.tile([128, DC, BL], F32)
        nc.scalar.dma_start(decT_sb[:], decT.rearrange("(c p) b -> p c b", p=128))
        bdec_sb = cpool.tile([128, UC], F32)
        nc.scalar.dma_start(bdec_sb[:], bdec[:])
        wcov_sb = cpool.tile([128, UC], F32)
        nc.scalar.dma_start(wcov_sb[:], wcov[:])
        watt_sb = cpool.tile([128, UC], BF16)
        nc.scalar.dma_start(watt_sb[:], watt[:])

        ident = cpool.tile([1, 1], F32)
        nc.vector.memset(ident[:], 1.0)

        # dec_attnT[u, b] = sum_d W_dec[d, u] * dec_state[b, d]   (+ b_dec)
        psum_dec = pmisc_pool.tile([128, UC * BL], F32, tag="misc", name="psum_dec")
        for uc in range(UC):
            for dc in range(DC):
                nc.tensor.matmul(
                    psum_dec[:, uc * BL : (uc + 1) * BL],
                    lhsT=wdec_sb[:, dc, uc * 128 : (uc + 1) * 128],
                    rhs=decT_sb[:, dc, :],
                    start=(dc == 0),
                    stop=(dc == DC - 1),
                )
        dec_sb = cpool.tile([128, UC * BL], F32)
        for uc in range(UC):
            nc.vector.tensor_scalar(
                dec_sb[:, uc * BL : (uc + 1) * BL],
                in0=psum_dec[:, uc * BL : (uc + 1) * BL],
                scalar1=bdec_sb[:, uc : uc + 1],
                scalar2=None,
                op0=ALU.add,
            )

        warm_ps = pwarm_pool.tile([1, 512], F32, name="warm_ps")

        def pe_warm(n):
            for _ in range(n):
                nc.tensor.matmul(
                    warm_ps[:],
                    lhsT=cov_all_rep[:, 0:1],
                    rhs=cov_all_rep[:, 0:512],
                    start=True,
                    stop=True,
                    skip_group_check=True,
                )

        # ---- main loop over local batch ----
        for b in range(BL):
            # phase A: scores
            encT_t = encpool.tile([128, UC, S], BF16, tag="encT", name="encT_t")
            nc.sync.dma_start(encT_t[:], encT[b].rearrange("(c p) s -> p c s", p=128))
            psum_sc = psc_pool.tile([1, S], F32, name="psum_sc")
            for uc in range(UC):
                covw = wpool.tile([128, S], BF16, tag="covw", name="covw")
                nc.vector.tensor_scalar(
                    covw[:],
                    in0=cov_reps[b],
                    scalar1=wcov_sb[:, uc : uc + 1],
                    scalar2=None,
                    op0=ALU.mult,
                )
                z = wpool.tile([128, S], BF16, tag="z", name="z")
                add_eng = nc.gpsimd if uc in GP_ADD_UC else nc.vector
                add_eng.tensor_tensor(z[:], covw[:], encT_t[:, uc, :], op=ALU.add)
                t = wpool.tile([128, S], BF16, tag="t", name="t")
                nc.scalar.activation(
                    t[:], z[:], ACTF.Tanh,
                    bias=dec_sb[:, uc * BL + b : uc * BL + b + 1],
                )
                for sl in range(S // 512):
                    nc.tensor.matmul(
                        psum_sc[0:1, sl * 512 : (sl + 1) * 512],
                        lhsT=watt_sb[:, uc : uc + 1],
                        rhs=t[:, sl * 512 : (sl + 1) * 512],
                        start=(uc == 0),
                        stop=(uc == UC - 1),
                        skip_group_check=True,
                    )
                pe_warm(WARM_PER_UC)

            # phase B: softmax (no max subtraction; scores are bounded)
            escore = spool.tile([1, S], F32, tag="escore", name="escore")
            total = spool.tile([1, 1], F32, tag="total", name="total")
            nc.scalar.activation(
                escore[:], psum_sc[0:1, :], ACTF.Exp, accum_out=total[:]
            )
            recip = spool.tile([1, 1], F32, tag="recip", name="recip")
            nc.vector.reciprocal(recip[:], total[:])
            attn_row = spool.tile([1, S], F32, tag="attnrow", name="attn_row")
            nc.vector.tensor_scalar(
                attn_row[:], in0=escore[:], scalar1=recip[:], scalar2=None,
                op0=ALU.mult,
            )
            nc.sync.dma_start(attn[b : b + 1, :], attn_row[:])

            # transpose attn row into [128, SC] (s on partitions)
            psum_T = pmisc_pool.tile([128, SC], F32, tag="misc", name="psum_T")
            for c in range(SC):
                nc.tensor.transpose(
                    psum_T[:, c : c + 1],
                    attn_row[0:1, c * 128 : (c + 1) * 128],
                    ident[:],
                )
            attn_T = spool.tile([128, SC], BF16, tag="attnT", name="attn_T")
            nc.vector.tensor_copy(attn_T[:], psum_T[:])

            # phase C: context vector
            enc_t = encpool.tile([128, SC, H], BF16, tag="enc", name="enc_t")
            nc.sync.dma_start(enc_t[:], enc[b].rearrange("(c p) h -> p c h", p=128))
            psum_c = pc_pool.tile([1, H], F32, name="psum_c")
            for c in range(SC):
                nc.tensor.matmul(
                    psum_c[0:1, :],
                    lhsT=attn_T[:, c : c + 1],
                    rhs=enc_t[:, c, :],
                    start=(c == 0),
                    stop=(c == SC - 1),
                )
            pe_warm(WARM_PER_B)
            cvec_row = spool.tile([1, H], F32, tag="cvecrow", name="cvec_row")
            nc.scalar.copy(cvec_row[:], psum_c[0:1, :])
            nc.sync.dma_start(cvec[b : b + 1, :], cvec_row[:])


_NC = None


def build_nc():
    global _NC
    if _NC is not None:
        return _NC
    nc = bacc.Bacc("TRN2", target_bir_lowering=False, debug=False)
    aps = {}
    for name, shape in [
        ("decT", [D, BL]),
        ("wdec", [D, U]),
        ("bdec", [128, UC]),
        ("wcov", [128, UC]),
    ]:
        aps[name] = nc.dram_tensor(name, shape, F32, kind="ExternalInput").ap()
    for name, shape in [
        ("cov", [BL, S]),
        ("enc", [BL, S, H]),
        ("encT", [BL, U, S]),
        ("watt", [128, UC]),
    ]:
        aps[name] = nc.dram_tensor(name, shape, BF16, kind="ExternalInput").ap()
    for name, shape in [("attn", [BL, S]), ("cvec", [BL, H])]:
        aps[name] = nc.dram_tensor(name, shape, F32, kind="ExternalOutput").ap()

    with tile.TileContext(nc) as tc:
        _build_body(tc, aps)
    nc.compile()
    _NC = nc
    return nc


def make_in_maps(enc_output, enc_attn, coverage_vector, dec_state, W_dec, b_dec,
                 W_cov, w_attn):
    enc_output = np.asarray(enc_output, dtype=np.float32)
    enc_attn = np.asarray(enc_attn, dtype=np.float32)
    coverage_vector = np.asarray(coverage_vector, dtype=np.float32).astype(
        ml_dtypes.bfloat16
    )
    dec_state = np.asarray(dec_state, dtype=np.float32)
    wdec = np.ascontiguousarray(np.asarray(W_dec, dtype=np.float32))
    bdec = np.ascontiguousarray(np.asarray(b_dec, dtype=np.float32).reshape(UC, 128).T)
    wcov = np.ascontiguousarray(
        np.asarray(W_cov, dtype=np.float32)[0].reshape(UC, 128).T
    )
    watt = np.ascontiguousarray(
        np.asarray(w_attn, dtype=np.float32)[:, 0].reshape(UC, 128).T
    ).astype(ml_dtypes.bfloat16)
    in_maps = []
    for i in range(NCORES):
        sl = slice(i * BL, (i + 1) * BL)
        in_maps.append({
            "encT": np.ascontiguousarray(
                enc_attn[sl].transpose(0, 2, 1)
            ).astype(ml_dtypes.bfloat16),
            "enc": np.ascontiguousarray(enc_output[sl]).astype(ml_dtypes.bfloat16),
            "cov": np.ascontiguousarray(coverage_vector[sl]),
            "decT": np.ascontiguousarray(dec_state[sl].T),
            "wdec": wdec,
            "bdec": bdec,
            "wcov": wcov,
            "watt": watt,
        })
    return in_maps


def kernel(enc_output, enc_attn, coverage_vector, dec_state, W_dec, b_dec,
           W_cov, w_attn, **run_kwargs):
    global LAST_RESULTS
    nc = build_nc()
    in_maps = make_in_maps(enc_output, enc_attn, coverage_vector, dec_state,
                           W_dec, b_dec, W_cov, w_attn)
    res = bass_utils.run_bass_kernel_spmd(
        nc, in_maps, core_ids=list(range(NCORES)), **run_kwargs
    )
    LAST_RESULTS = res
    attn = np.concatenate([r["attn"] for r in res.results], axis=0)
    cvec = np.concatenate([r["cvec"] for r in res.results], axis=0)
    return attn.reshape(B, S, 1), cvec


# revision 8
# speedup vs baseline: 1.5846x; 1.0584x over previous
"""Bahdanau attention with coverage — Trainium2 Bass kernel.

Full-input contract: kernel(**inputs) takes the unsharded numpy inputs,
shards batch B=64 across 8 NeuronCores (8 batch elements per core),
runs one SPMD Bass kernel, and reassembles the full outputs.

Math per batch element i (S=1024 source positions, U=H=D=512):
    dec_attn = dec_state[i] @ W_dec + b_dec                  # [U]
    z[s,u]   = enc_attn[i,s,u] + coverage[i,s]*W_cov[u] + dec_attn[u]
    scores_s = sum_u tanh(z[s,u]) * w_attn[u]                # [S]
    attn     = softmax(scores)                               # [S]
    c        = sum_s attn_s * enc_output[i,s,:]              # [H]

Device mapping (per core, per batch element b of 8):
  * enc_attn is pre-transposed on host to [U, S] so U sits on SBUF
    partitions.  Then per u-chunk (128 partitions):
      - DVE tensor_scalar: zc = cov_bcast * W_cov[p] + dec_attn[p]
        (coverage row replicated across partitions once per b via GPSIMD)
      - DVE tensor_tensor: z = zc + encT_tile
      - ACT: t = tanh(z)
      - PE:  scores[1,S] += w_chunk.T @ t     (fp32r, PSUM accumulate)
  * softmax: ACT exp with accum_out (free-dim total), DVE reciprocal,
    DVE tensor_scalar scale.  No max-subtraction: |scores| <= sum|w| ~ 18,
    fp32 exp is exact-safe there and softmax is shift-invariant.
  * context: PE transposes attn row into [128,1] chunks (via identity
    matmul) and accumulates attn.T @ enc_output tiles into PSUM [1,H].
"""

import ml_dtypes
import numpy as np

import concourse.bacc as bacc
import concourse.mybir as mybir
import concourse.tile as tile
from concourse import bass_utils

F32 = mybir.dt.float32
F32R = mybir.dt.float32r
BF16 = mybir.dt.bfloat16
ALU = mybir.AluOpType
ACTF = mybir.ActivationFunctionType

B, S, U, H, D = 64, 1024, 512, 512, 512
NCORES = 8
BL = B // NCORES          # batch per core
UC = U // 128             # u chunks per core
SC = S // 128             # s chunks
DC = D // 128             # d chunks

# u-chunks whose z-add runs on GpSimd instead of DVE
GP_ADD_UC = set()

# PE HAM warmer: dummy matmuls interleaved to keep the clock at 2.4 GHz
WARM_PER_UC = 2
WARM_PER_B = 4

# test.py reads these for profiling
LAST_RESULTS = None


def _build_body(tc, aps):
    nc = tc.nc
    encT = aps["encT"]      # [BL, U, S]
    enc = aps["enc"]        # [BL, S, H]
    cov = aps["cov"]        # [BL, S]
    decT = aps["decT"]      # [D, BL]
    wdec = aps["wdec"]      # [D, U]
    bdec = aps["bdec"]      # [128, UC]
    wcov = aps["wcov"]      # [128, UC]
    watt = aps["watt"]      # [128, UC]
    attn = aps["attn"]      # [BL, S]  out
    cvec = aps["cvec"]      # [BL, H]  out

    with (
        tc.tile_pool(name="const", bufs=1) as cpool,
        tc.tile_pool(name="encs", bufs=3) as encpool,
        tc.tile_pool(name="work", bufs=4) as wpool,
        tc.tile_pool(name="small", bufs=2) as spool,
        tc.tile_pool(name="psc", bufs=2, space="PSUM") as psc_pool,
        tc.tile_pool(name="pmisc", bufs=1, space="PSUM") as pmisc_pool,
        tc.tile_pool(name="pc", bufs=2, space="PSUM") as pc_pool,
        tc.tile_pool(name="pwarm", bufs=1, space="PSUM") as pwarm_pool,
    ):
        # ---- constants / setup ----
        # coverage first: each broadcast gates only its own batch's z chain
        cov_rows = []
        cov_reps = []
        for b in range(BL):
            cov_row = cpool.tile([1, S], BF16, tag=f"covrow{b}", name=f"covrow{b}")
            nc.scalar.dma_start(cov_row[:], cov[b : b + 1, :])
            cov_rows.append(cov_row)
        for b in range(BL):
            cov_rep = cpool.tile([128, S], BF16, tag=f"covrep{b}", name=f"covrep{b}")
            nc.gpsimd.partition_broadcast(cov_rep[:], cov_rows[b][:])
            cov_reps.append(cov_rep)

        wdec_sb = cpool.tile([128, DC, U], F32)
        nc.scalar.dma_start(wdec_sb[:], wdec.rearrange("(c p) u -> p c u", p=128))
        decT_sb = cpool.tile([128, DC, BL], F32)
        nc.scalar.dma_start(decT_sb[:], decT.rearrange("(c p) b -> p c b", p=128))
        bdec_sb = cpool.tile([128, UC], F32)
        nc.scalar.dma_start(bdec_sb[:], bdec[:])
        wcov_sb = cpool.tile([128, UC], F32)
        nc.scalar.dma_start(wcov_sb[:], wcov[:])
        watt_sb = cpool.tile([128, UC], BF16)
        nc.scalar.dma_start(watt_sb[:], watt[:])

        ident = cpool.tile([1, 1], F32)
        nc.vector.memset(ident[:], 1.0)

        # dec_attnT[u, b] = sum_d W_dec[d, u] * dec_state[b, d]   (+ b_dec)
        psum_dec = pmisc_pool.tile([128, UC * BL], F32, tag="misc", name="psum_dec")
        for uc in range(UC):
            for dc in range(DC):
                nc.tensor.matmul(
                    psum_dec[:, uc * BL : (uc + 1) * BL],
                    lhsT=wdec_sb[:, dc, uc * 128 : (uc + 1) * 128],
                    rhs=decT_sb[:, dc, :],
                    start=(dc == 0),
                    stop=(dc == DC - 1),
                )
        dec_sb = cpool.tile([128, UC * BL], F32)
        for uc in range(UC):
            nc.vector.tensor_scalar(
                dec_sb[:, uc * BL : (uc + 1) * BL],
                in0=psum_dec[:, uc * BL : (uc + 1) * BL],
                scalar1=bdec_sb[:, uc : uc + 1],
                scalar2=None,
                op0=ALU.add,
            )

        warm_ps = pwarm_pool.tile([1, 512], F32, name="warm_ps")

        def pe_warm(n):
            for _ in range(n):
                nc.tensor.matmul(
                    warm_ps[:],
                    lhsT=cov_reps[0][:, 0:1],
                    rhs=cov_reps[0][:, 0:512],
                    start=True,
                    stop=True,
                    skip_group_check=True,
                )

        # ---- main loop over local batch ----
        for b in range(BL):
            # phase A: scores
            encT_t = encpool.tile([128, UC, S], BF16, tag="encT", name="encT_t")
            nc.sync.dma_start(encT_t[:], encT[b].rearrange("(c p) s -> p c s", p=128))
            psum_sc = psc_pool.tile([1, S], F32, name="psum_sc")
            for uc in range(UC):
                covw = wpool.tile([128, S], BF16, tag="covw", name="covw")
                nc.vector.tensor_scalar(
                    covw[:],
                    in0=cov_reps[b],
                    scalar1=wcov_sb[:, uc : uc + 1],
                    scalar2=None,
                    op0=ALU.mult,
                )
                z = wpool.tile([128, S], BF16, tag="z", name="z")
                add_eng = nc.gpsimd if uc in GP_ADD_UC else nc.vector
                add_eng.tensor_tensor(z[:], covw[:], encT_t[:, uc, :], op=ALU.add)
                t = wpool.tile([128, S], BF16, tag="t", name="t")
                nc.scalar.activation(
                    t[:], z[:], ACTF.Tanh,
                    bias=dec_sb[:, uc * BL + b : uc * BL + b + 1],
                )
                for sl in range(S // 512):
                    nc.tensor.matmul(
                        psum_sc[0:1, sl * 512 : (sl + 1) * 512],
                        lhsT=watt_sb[:, uc : uc + 1],
                        rhs=t[:, sl * 512 : (sl + 1) * 512],
                        start=(uc == 0),
                        stop=(uc == UC - 1),
                        skip_group_check=True,
                    )
                pe_warm(WARM_PER_UC)

            # phase B: softmax (no max subtraction; scores are bounded)
            escore = spool.tile([1, S], F32, tag="escore", name="escore")
            total = spool.tile([1, 1], F32, tag="total", name="total")
            nc.scalar.activation(
                escore[:], psum_sc[0:1, :], ACTF.Exp, accum_out=total[:]
            )
            recip = spool.tile([1, 1], F32, tag="recip", name="recip")
            nc.vector.reciprocal(recip[:], total[:])
            attn_row = spool.tile([1, S], F32, tag="attnrow", name="attn_row")
            nc.vector.tensor_scalar(
                attn_row[:], in0=escore[:], scalar1=recip[:], scalar2=None,
                op0=ALU.mult,
            )
            nc.sync.dma_start(attn[b : b + 1, :], attn_row[:])

            # transpose attn row into [128, SC] (s on partitions)
            psum_T = pmisc_pool.tile([128, SC], F32, tag="misc", name="psum_T")
            for c in range(SC):
                nc.tensor.transpose(
                    psum_T[:, c : c + 1],
                    attn_row[0:1, c * 128 : (c + 1) * 128],
                    ident[:],
                )
            attn_T = spool.tile([128, SC], BF16, tag="attnT", name="attn_T")
            nc.vector.tensor_copy(attn_T[:], psum_T[:])

            # phase C: context vector
            enc_t = encpool.tile([128, SC, H], BF16, tag="enc", name="enc_t")
            nc.sync.dma_start(enc_t[:], enc[b].rearrange("(c p) h -> p c h", p=128))
            psum_c = pc_pool.tile([1, H], F32, name="psum_c")
            for c in range(SC):
                nc.tensor.matmul(
                    psum_c[0:1, :],
                    lhsT=attn_T[:, c : c + 1],
                    rhs=enc_t[:, c, :],
                    start=(c == 0),
                    stop=(c == SC - 1),
                )
            pe_warm(WARM_PER_B)
            cvec_row = spool.tile([1, H], F32, tag="cvecrow", name="cvec_row")
            nc.scalar.copy(cvec_row[:], psum_c[0:1, :])
            nc.sync.dma_start(cvec[b : b + 1, :], cvec_row[:])


_NC = None


def build_nc():
    global _NC
    if _NC is not None:
        return _NC
    nc = bacc.Bacc("TRN2", target_bir_lowering=False, debug=False)
    aps = {}
    for name, shape in [
        ("decT", [D, BL]),
        ("wdec", [D, U]),
        ("bdec", [128, UC]),
        ("wcov", [128, UC]),
    ]:
        aps[name] = nc.dram_tensor(name, shape, F32, kind="ExternalInput").ap()
    for name, shape in [
        ("cov", [BL, S]),
        ("enc", [BL, S, H]),
        ("encT", [BL, U, S]),
        ("watt", [128, UC]),
    ]:
        aps[name] = nc.dram_tensor(name, shape, BF16, kind="ExternalInput").ap()
    for name, shape in [("attn", [BL, S]), ("cvec", [BL, H])]:
        aps[name] = nc.dram_tensor(name, shape, F32, kind="ExternalOutput").ap()

    with tile.TileContext(nc) as tc:
        _build_body(tc, aps)
    nc.compile()
    _NC = nc
    return nc


def make_in_maps(enc_output, enc_attn, coverage_vector, dec_state, W_dec, b_dec,
                 W_cov, w_attn):
    enc_output = np.asarray(enc_output, dtype=np.float32)
    enc_attn = np.asarray(enc_attn, dtype=np.float32)
    coverage_vector = np.asarray(coverage_vector, dtype=np.float32).astype(
        ml_dtypes.bfloat16
    )
    dec_state = np.asarray(dec_state, dtype=np.float32)
    wdec = np.ascontiguousarray(np.asarray(W_dec, dtype=np.float32))
    bdec = np.ascontiguousarray(np.asarray(b_dec, dtype=np.float32).reshape(UC, 128).T)
    wcov = np.ascontiguousarray(
        np.asarray(W_cov, dtype=np.float32)[0].reshape(UC, 128).T
    )
    watt = np.ascontiguousarray(
        np.asarray(w_attn, dtype=np.float32)[:, 0].reshape(UC, 128).T
    ).astype(ml_dtypes.bfloat16)
    in_maps = []
    for i in range(NCORES):
        sl = slice(i * BL, (i + 1) * BL)
        in_maps.append({
            "encT": np.ascontiguousarray(
                enc_attn[sl].transpose(0, 2, 1)
            ).astype(ml_dtypes.bfloat16),
            "enc": np.ascontiguousarray(enc_output[sl]).astype(ml_dtypes.bfloat16),
            "cov": np.ascontiguousarray(coverage_vector[sl]),
            "decT": np.ascontiguousarray(dec_state[sl].T),
            "wdec": wdec,
            "bdec": bdec,
            "wcov": wcov,
            "watt": watt,
        })
    return in_maps


def kernel(enc_output, enc_attn, coverage_vector, dec_state, W_dec, b_dec,
           W_cov, w_attn, **run_kwargs):
    global LAST_RESULTS
    nc = build_nc()
    in_maps = make_in_maps(enc_output, enc_attn, coverage_vector, dec_state,
                           W_dec, b_dec, W_cov, w_attn)
    res = bass_utils.run_bass_kernel_spmd(
        nc, in_maps, core_ids=list(range(NCORES)), **run_kwargs
    )
    LAST_RESULTS = res
    attn = np.concatenate([r["attn"] for r in res.results], axis=0)
    cvec = np.concatenate([r["cvec"] for r in res.results], axis=0)
    return attn.reshape(B, S, 1), cvec


# revision 9
# speedup vs baseline: 1.6219x; 1.0236x over previous
"""Bahdanau attention with coverage — Trainium2 Bass kernel.

Full-input contract: kernel(**inputs) takes the unsharded numpy inputs,
shards batch B=64 across 8 NeuronCores (8 batch elements per core),
runs one SPMD Bass kernel, and reassembles the full outputs.

Math per batch element i (S=1024 source positions, U=H=D=512):
    dec_attn = dec_state[i] @ W_dec + b_dec                  # [U]
    z[s,u]   = enc_attn[i,s,u] + coverage[i,s]*W_cov[u] + dec_attn[u]
    scores_s = sum_u tanh(z[s,u]) * w_attn[u]                # [S]
    attn     = softmax(scores)                               # [S]
    c        = sum_s attn_s * enc_output[i,s,:]              # [H]

Device mapping (per core, per batch element b of 8):
  * enc_attn is pre-transposed on host to [U, S] so U sits on SBUF
    partitions.  Then per u-chunk (128 partitions):
      - DVE tensor_scalar: zc = cov_bcast * W_cov[p] + dec_attn[p]
        (coverage row replicated across partitions once per b via GPSIMD)
      - DVE tensor_tensor: z = zc + encT_tile
      - ACT: t = tanh(z)
      - PE:  scores[1,S] += w_chunk.T @ t     (fp32r, PSUM accumulate)
  * softmax: ACT exp with accum_out (free-dim total), DVE reciprocal,
    DVE tensor_scalar scale.  No max-subtraction: |scores| <= sum|w| ~ 18,
    fp32 exp is exact-safe there and softmax is shift-invariant.
  * context: PE transposes attn row into [128,1] chunks (via identity
    matmul) and accumulates attn.T @ enc_output tiles into PSUM [1,H].
"""

import ml_dtypes
import numpy as np

import concourse.bacc as bacc
import concourse.mybir as mybir
import concourse.tile as tile
from concourse import bass_utils

F32 = mybir.dt.float32
F32R = mybir.dt.float32r
BF16 = mybir.dt.bfloat16
ALU = mybir.AluOpType
ACTF = mybir.ActivationFunctionType

B, S, U, H, D = 64, 1024, 512, 512, 512
NCORES = 8
BL = B // NCORES          # batch per core
UC = U // 128             # u chunks per core
SC = S // 128             # s chunks
DC = D // 128             # d chunks

# u-chunks whose z-add runs on GpSimd instead of DVE
GP_ADD_UC = set()

# PE HAM warmer: dummy matmuls interleaved to keep the clock at 2.4 GHz
WARM_PER_UC = 2
WARM_PER_B = 4

# test.py reads these for profiling
LAST_RESULTS = None


def _build_body(tc, aps):
    nc = tc.nc
    encT = aps["encT"]      # [BL, U, S]
    enc = aps["enc"]        # [BL, S, H]
    cov = aps["cov"]        # [BL, S]
    decT = aps["decT"]      # [D, BL]
    wdec = aps["wdec"]      # [D, U]
    bdec = aps["bdec"]      # [128, UC]
    wcov = aps["wcov"]      # [128, UC]
    watt = aps["watt"]      # [128, UC]
    attn = aps["attn"]      # [BL, S]  out
    cvec = aps["cvec"]      # [BL, H]  out

    with (
        tc.tile_pool(name="const", bufs=1) as cpool,
        tc.tile_pool(name="encs", bufs=4) as encpool,
        tc.tile_pool(name="work", bufs=4) as wpool,
        tc.tile_pool(name="small", bufs=3) as spool,
        tc.tile_pool(name="psc", bufs=2, space="PSUM") as psc_pool,
        tc.tile_pool(name="pmisc", bufs=1, space="PSUM") as pmisc_pool,
        tc.tile_pool(name="pc", bufs=2, space="PSUM") as pc_pool,
        tc.tile_pool(name="pwarm", bufs=1, space="PSUM") as pwarm_pool,
    ):
        # ---- constants / setup ----
        # coverage first: one DMA, then per-b broadcasts (each gates only
        # its own batch's z chain)
        cov_all_row = cpool.tile([1, BL * S], BF16)
        nc.scalar.dma_start(cov_all_row[:], cov.rearrange("b s -> (b s)").unsqueeze(0))
        cov_reps = []
        for b in range(BL):
            cov_rep = cpool.tile([128, S], BF16, tag=f"covrep{b}", name=f"covrep{b}")
            nc.gpsimd.partition_broadcast(
                cov_rep[:], cov_all_row[:, b * S : (b + 1) * S]
            )
            cov_reps.append(cov_rep)

        wdec_sb = cpool.tile([128, DC, U], F32)
        nc.scalar.dma_start(wdec_sb[:], wdec.rearrange("(c p) u -> p c u", p=128))
        decT_sb = cpool.tile([128, DC, BL], F32)
        nc.scalar.dma_start(decT_sb[:], decT.rearrange("(c p) b -> p c b", p=128))
        bdec_sb = cpool.tile([128, UC], F32)
        nc.scalar.dma_start(bdec_sb[:], bdec[:])
        wcov_sb = cpool.tile([128, UC], F32)
        nc.scalar.dma_start(wcov_sb[:], wcov[:])
        watt_sb = cpool.tile([128, UC], BF16)
        nc.scalar.dma_start(watt_sb[:], watt[:])

        ident = cpool.tile([1, 1], F32)
        nc.vector.memset(ident[:], 1.0)

        # dec_attnT[u, b] = sum_d W_dec[d, u] * dec_state[b, d]   (+ b_dec)
        psum_dec = pmisc_pool.tile([128, UC * BL], F32, tag="misc", name="psum_dec")
        for uc in range(UC):
            for dc in range(DC):
                nc.tensor.matmul(
                    psum_dec[:, uc * BL : (uc + 1) * BL],
                    lhsT=wdec_sb[:, dc, uc * 128 : (uc + 1) * 128],
                    rhs=decT_sb[:, dc, :],
                    start=(dc == 0),
                    stop=(dc == DC - 1),
                )
        dec_sb = cpool.tile([128, UC * BL], F32)
        for uc in range(UC):
            nc.vector.tensor_scalar(
                dec_sb[:, uc * BL : (uc + 1) * BL],
                in0=psum_dec[:, uc * BL : (uc + 1) * BL],
                scalar1=bdec_sb[:, uc : uc + 1],
                scalar2=None,
                op0=ALU.add,
            )

        warm_ps = pwarm_pool.tile([1, 512], F32, name="warm_ps")

        def pe_warm(n):
            for _ in range(n):
                nc.tensor.matmul(
                    warm_ps[:],
                    lhsT=cov_reps[0][:, 0:1],
                    rhs=cov_reps[0][:, 0:512],
                    start=True,
                    stop=True,
                    skip_group_check=True,
                )

        # ---- main loop over local batch ----
        for b in range(BL):
            # phase A: scores
            encT_t = encpool.tile([128, UC, S], BF16, tag="encT", name="encT_t")
            nc.sync.dma_start(encT_t[:], encT[b].rearrange("(c p) s -> p c s", p=128))
            psum_sc = psc_pool.tile([1, S], F32, name="psum_sc")
            for uc in range(UC):
                covw = wpool.tile([128, S], BF16, tag="covw", name="covw", bufs=8)
                nc.vector.tensor_scalar(
                    covw[:],
                    in0=cov_reps[b],
                    scalar1=wcov_sb[:, uc : uc + 1],
                    scalar2=None,
                    op0=ALU.mult,
                )
                z = wpool.tile([128, S], BF16, tag="z", name="z", bufs=8)
                add_eng = nc.gpsimd if uc in GP_ADD_UC else nc.vector
                add_eng.tensor_tensor(z[:], covw[:], encT_t[:, uc, :], op=ALU.add)
                t = wpool.tile([128, S], BF16, tag="t", name="t", bufs=8)
                nc.scalar.activation(
                    t[:], z[:], ACTF.Tanh,
                    bias=dec_sb[:, uc * BL + b : uc * BL + b + 1],
                )
                for sl in range(S // 512):
                    nc.tensor.matmul(
                        psum_sc[0:1, sl * 512 : (sl + 1) * 512],
                        lhsT=watt_sb[:, uc : uc + 1],
                        rhs=t[:, sl * 512 : (sl + 1) * 512],
                        start=(uc == 0),
                        stop=(uc == UC - 1),
                        skip_group_check=True,
                    )
                pe_warm(WARM_PER_UC)

            # phase B: softmax (no max subtraction; scores are bounded)
            escore = spool.tile([1, S], F32, tag="escore", name="escore")
            total = spool.tile([1, 1], F32, tag="total", name="total")
            nc.scalar.activation(
                escore[:], psum_sc[0:1, :], ACTF.Exp, accum_out=total[:]
            )
            recip = spool.tile([1, 1], F32, tag="recip", name="recip")
            nc.vector.reciprocal(recip[:], total[:])
            attn_row = spool.tile([1, S], F32, tag="attnrow", name="attn_row")
            nc.vector.tensor_scalar(
                attn_row[:], in0=escore[:], scalar1=recip[:], scalar2=None,
                op0=ALU.mult,
            )
            nc.sync.dma_start(attn[b : b + 1, :], attn_row[:])

            pe_warm(WARM_PER_B)
            # transpose attn row into [128, SC] (s on partitions)
            psum_T = pmisc_pool.tile([128, SC], F32, tag="misc", name="psum_T")
            for c in range(SC):
                nc.tensor.transpose(
                    psum_T[:, c : c + 1],
                    attn_row[0:1, c * 128 : (c + 1) * 128],
                    ident[:],
                )
            attn_T = spool.tile([128, SC], BF16, tag="attnT", name="attn_T")
            nc.vector.tensor_copy(attn_T[:], psum_T[:])

            # phase C: context vector
            enc_t = encpool.tile([128, SC, H], BF16, tag="enc", name="enc_t")
            nc.sync.dma_start(enc_t[:], enc[b].rearrange("(c p) h -> p c h", p=128))
            psum_c = pc_pool.tile([1, H], F32, name="psum_c")
            for c in range(SC):
                nc.tensor.matmul(
                    psum_c[0:1, :],
                    lhsT=attn_T[:, c : c + 1],
                    rhs=enc_t[:, c, :],
                    start=(c == 0),
                    stop=(c == SC - 1),
                )
            pe_warm(WARM_PER_B)
            cvec_row = spool.tile([1, H], F32, tag="cvecrow", name="cvec_row")
            nc.scalar.copy(cvec_row[:], psum_c[0:1, :])
            nc.sync.dma_start(cvec[b : b + 1, :], cvec_row[:])


_NC = None


def build_nc():
    global _NC
    if _NC is not None:
        return _NC
    nc = bacc.Bacc("TRN2", target_bir_lowering=False, debug=False)
    aps = {}
    for name, shape in [
        ("decT", [D, BL]),
        ("wdec", [D, U]),
        ("bdec", [128, UC]),
        ("wcov", [128, UC]),
    ]:
        aps[name] = nc.dram_tensor(name, shape, F32, kind="ExternalInput").ap()
    for name, shape in [
        ("cov", [BL, S]),
        ("enc", [BL, S, H]),
        ("encT", [BL, U, S]),
        ("watt", [128, UC]),
    ]:
        aps[name] = nc.dram_tensor(name, shape, BF16, kind="ExternalInput").ap()
    for name, shape in [("attn", [BL, S]), ("cvec", [BL, H])]:
        aps[name] = nc.dram_tensor(name, shape, F32, kind="ExternalOutput").ap()

    with tile.TileContext(nc) as tc:
        _build_body(tc, aps)
    nc.compile()
    _NC = nc
    return nc


def make_in_maps(enc_output, enc_attn, coverage_vector, dec_state, W_dec, b_dec,
                 W_cov, w_attn):
    enc_output = np.asarray(enc_output, dtype=np.float32)
    enc_attn = np.asarray(enc_attn, dtype=np.float32)
    coverage_vector = np.asarray(coverage_vector, dtype=np.float32).astype(
        ml_dtypes.bfloat16
    )
    dec_state = np.asarray(dec_state, dtype=np.float32)
    wdec = np.ascontiguousarray(np.asarray(W_dec, dtype=np.float32))
    bdec = np.ascontiguousarray(np.asarray(b_dec, dtype=np.float32).reshape(UC, 128).T)
    wcov = np.ascontiguousarray(
        np.asarray(W_cov, dtype=np.float32)[0].reshape(UC, 128).T
    )
    watt = np.ascontiguousarray(
        np.asarray(w_attn, dtype=np.float32)[:, 0].reshape(UC, 128).T
    ).astype(ml_dtypes.bfloat16)
    in_maps = []
    for i in range(NCORES):
        sl = slice(i * BL, (i + 1) * BL)
        in_maps.append({
            "encT": np.ascontiguousarray(
                enc_attn[sl].transpose(0, 2, 1)
            ).astype(ml_dtypes.bfloat16),
            "enc": np.ascontiguousarray(enc_output[sl]).astype(ml_dtypes.bfloat16),
            "cov": np.ascontiguousarray(coverage_vector[sl]),
            "decT": np.ascontiguousarray(dec_state[sl].T),
            "wdec": wdec,
            "bdec": bdec,
            "wcov": wcov,
            "watt": watt,
        })
    return in_maps


def kernel(enc_output, enc_attn, coverage_vector, dec_state, W_dec, b_dec,
           W_cov, w_attn, **run_kwargs):
    global LAST_RESULTS
    nc = build_nc()
    in_maps = make_in_maps(enc_output, enc_attn, coverage_vector, dec_state,
                           W_dec, b_dec, W_cov, w_attn)
    res = bass_utils.run_bass_kernel_spmd(
        nc, in_maps, core_ids=list(range(NCORES)), **run_kwargs
    )
    LAST_RESULTS = res
    attn = np.concatenate([r["attn"] for r in res.results], axis=0)
    cvec = np.concatenate([r["cvec"] for r in res.results], axis=0)
    return attn.reshape(B, S, 1), cvec


# revision 14
# speedup vs baseline: 1.7322x; 1.0680x over previous
"""Bahdanau attention with coverage — Trainium2 Bass kernel.

Full-input contract: kernel(**inputs) takes the unsharded numpy inputs,
shards batch B=64 across 8 NeuronCores (8 batch elements per core),
runs one SPMD Bass kernel, and reassembles the full outputs.

Math per batch element i (S=1024 source positions, U=H=D=512):
    dec_attn = dec_state[i] @ W_dec + b_dec                  # [U]
    z[s,u]   = enc_attn[i,s,u] + coverage[i,s]*W_cov[u] + dec_attn[u]
    scores_s = sum_u tanh(z[s,u]) * w_attn[u]                # [S]
    attn     = softmax(scores)                               # [S]
    c        = sum_s attn_s * enc_output[i,s,:]              # [H]

Device mapping (per core, per batch element b of 8):
  * enc_attn is pre-transposed on host to [U, S] so U sits on SBUF
    partitions.  Then per u-chunk (128 partitions):
      - DVE tensor_scalar: zc = cov_bcast * W_cov[p] + dec_attn[p]
        (coverage row replicated across partitions once per b via GPSIMD)
      - DVE tensor_tensor: z = zc + encT_tile
      - ACT: t = tanh(z)
      - PE:  scores[1,S] += w_chunk.T @ t     (fp32r, PSUM accumulate)
  * softmax: ACT exp with accum_out (free-dim total), DVE reciprocal,
    DVE tensor_scalar scale.  No max-subtraction: |scores| <= sum|w| ~ 18,
    fp32 exp is exact-safe there and softmax is shift-invariant.
  * context: PE transposes attn row into [128,1] chunks (via identity
    matmul) and accumulates attn.T @ enc_output tiles into PSUM [1,H].
"""

import ml_dtypes
import numpy as np

import concourse.bacc as bacc
import concourse.mybir as mybir
import concourse.tile as tile
from concourse import bass_utils

F32 = mybir.dt.float32
F32R = mybir.dt.float32r
BF16 = mybir.dt.bfloat16
ALU = mybir.AluOpType
ACTF = mybir.ActivationFunctionType

B, S, U, H, D = 64, 1024, 512, 512, 512
NCORES = 8
BL = B // NCORES          # batch per core
UC = U // 128             # u chunks per core
SC = S // 128             # s chunks
DC = D // 128             # d chunks

# u-chunks whose z-add runs on GpSimd instead of DVE
GP_ADD_UC = set()

# PE HAM warmer: dummy matmuls interleaved to keep the clock at 2.4 GHz
WARM_PER_UC = 3
WARM_PER_B = 4

# test.py reads these for profiling
LAST_RESULTS = None


def _build_body(tc, aps):
    nc = tc.nc
    encT = aps["encT"]      # [BL, U, S]
    enc = aps["enc"]        # [BL, S, H]
    cov = aps["cov"]        # [BL, S]
    decT = aps["decT"]      # [D, BL]
    wdec = aps["wdec"]      # [D, U]
    bdec = aps["bdec"]      # [128, UC]
    wcov = aps["wcov"]      # [128, UC]
    watt = aps["watt"]      # [128, UC]
    attn = aps["attn"]      # [BL, S]  out
    cvec = aps["cvec"]      # [BL, H]  out

    with (
        tc.tile_pool(name="const", bufs=1) as cpool,
        tc.tile_pool(name="encs", bufs=4) as encpool,
        tc.tile_pool(name="work", bufs=4) as wpool,
        tc.tile_pool(name="small", bufs=3) as spool,
        tc.tile_pool(name="psc", bufs=2, space="PSUM") as psc_pool,
        tc.tile_pool(name="pmisc", bufs=1, space="PSUM") as pmisc_pool,
        tc.tile_pool(name="pc", bufs=2, space="PSUM") as pc_pool,
        tc.tile_pool(name="pwarm", bufs=1, space="PSUM") as pwarm_pool,
    ):
        # ---- constants / setup ----
        # coverage first: one DMA, then per-b broadcasts (each gates only
        # its own batch's z chain)
        cov_all_row = cpool.tile([1, BL * S], BF16)
        nc.scalar.dma_start(cov_all_row[:], cov.rearrange("b s -> (b s)").unsqueeze(0))
        cov_reps = []
        for b in range(BL):
            cov_rep = cpool.tile([128, S], BF16, tag=f"covrep{b}", name=f"covrep{b}")
            nc.gpsimd.partition_broadcast(
                cov_rep[:], cov_all_row[:, b * S : (b + 1) * S]
            )
            cov_reps.append(cov_rep)

        wdec_sb = cpool.tile([128, DC, U], F32)
        nc.scalar.dma_start(wdec_sb[:], wdec.rearrange("(c p) u -> p c u", p=128))
        decT_sb = cpool.tile([128, DC, BL], F32)
        nc.scalar.dma_start(decT_sb[:], decT.rearrange("(c p) b -> p c b", p=128))
        bdec_sb = cpool.tile([128, UC], F32)
        nc.scalar.dma_start(bdec_sb[:], bdec[:])
        wcov_sb = cpool.tile([128, UC], F32)
        nc.scalar.dma_start(wcov_sb[:], wcov[:])
        watt_sb = cpool.tile([128, UC], BF16)
        nc.scalar.dma_start(watt_sb[:], watt[:])

        ident = cpool.tile([1, 1], F32)
        nc.vector.memset(ident[:], 1.0)

        # dec_attnT[u, b] = sum_d W_dec[d, u] * dec_state[b, d]   (+ b_dec)
        psum_dec = pmisc_pool.tile([128, UC * BL], F32, tag="misc", name="psum_dec")
        for uc in range(UC):
            for dc in range(DC):
                nc.tensor.matmul(
                    psum_dec[:, uc * BL : (uc + 1) * BL],
                    lhsT=wdec_sb[:, dc, uc * 128 : (uc + 1) * 128],
                    rhs=decT_sb[:, dc, :],
                    start=(dc == 0),
                    stop=(dc == DC - 1),
                )
        dec_sb = cpool.tile([128, UC * BL], F32)
        for uc in range(UC):
            nc.vector.tensor_scalar(
                dec_sb[:, uc * BL : (uc + 1) * BL],
                in0=psum_dec[:, uc * BL : (uc + 1) * BL],
                scalar1=bdec_sb[:, uc : uc + 1],
                scalar2=None,
                op0=ALU.add,
            )

        warm_ps = pwarm_pool.tile([1, 512], F32, name="warm_ps")

        def pe_warm(n):
            for _ in range(n):
                nc.tensor.matmul(
                    warm_ps[:],
                    lhsT=cov_reps[0][:, 0:1],
                    rhs=cov_reps[0][:, 0:512],
                    start=True,
                    stop=True,
                    skip_group_check=True,
                )

        # ---- main loop over local batch ----
        for b in range(BL):
            # phase A: scores
            encT_t = encpool.tile([128, UC, S], BF16, tag="encT", name="encT_t")
            nc.sync.dma_start(encT_t[:], encT[b].rearrange("(c p) s -> p c s", p=128))
            psum_sc = psc_pool.tile([1, S], F32, name="psum_sc")
            for uc in range(UC):
                covw = wpool.tile([128, S], BF16, tag="covw", name="covw", bufs=8)
                nc.vector.tensor_scalar(
                    covw[:],
                    in0=cov_reps[b],
                    scalar1=wcov_sb[:, uc : uc + 1],
                    scalar2=None,
                    op0=ALU.mult,
                )
                z = wpool.tile([128, S], BF16, tag="z", name="z", bufs=8)
                add_eng = nc.gpsimd if uc in GP_ADD_UC else nc.vector
                add_eng.tensor_tensor(z[:], covw[:], encT_t[:, uc, :], op=ALU.add)
                t = wpool.tile([128, S], BF16, tag="t", name="t", bufs=8)
                nc.scalar.activation(
                    t[:], z[:], ACTF.Tanh,
                    bias=dec_sb[:, uc * BL + b : uc * BL + b + 1],
                )
                for sl in range(S // 512):
                    nc.tensor.matmul(
                        psum_sc[0:1, sl * 512 : (sl + 1) * 512],
                        lhsT=watt_sb[:, uc : uc + 1],
                        rhs=t[:, sl * 512 : (sl + 1) * 512],
                        start=(uc == 0),
                        stop=(uc == UC - 1),
                        skip_group_check=True,
                    )
                pe_warm(WARM_PER_UC)

            # phase B: softmax (no max subtraction; scores are bounded)
            escore = spool.tile([1, S], F32, tag="escore", name="escore")
            total = spool.tile([1, 1], F32, tag="total", name="total")
            nc.scalar.activation(
                escore[:], psum_sc[0:1, :], ACTF.Exp, accum_out=total[:]
            )
            recip = spool.tile([1, 1], F32, tag="recip", name="recip")
            nc.vector.reciprocal(recip[:], total[:])
            attn_row = spool.tile([1, S], F32, tag="attnrow", name="attn_row")
            nc.vector.tensor_scalar(
                attn_row[:], in0=escore[:], scalar1=recip[:], scalar2=None,
                op0=ALU.mult,
            )
            nc.sync.dma_start(attn[b : b + 1, :], attn_row[:])

            pe_warm(WARM_PER_B)
            # transpose attn row into [128, SC] (s on partitions)
            psum_T = pmisc_pool.tile([128, SC], F32, tag="misc", name="psum_T")
            for c in range(SC):
                nc.tensor.transpose(
                    psum_T[:, c : c + 1],
                    attn_row[0:1, c * 128 : (c + 1) * 128],
                    ident[:],
                )
            attn_T = spool.tile([128, SC], BF16, tag="attnT", name="attn_T")
            nc.vector.tensor_copy(attn_T[:], psum_T[:])

            # phase C: context vector
            enc_t = encpool.tile([128, SC, H], BF16, tag="enc", name="enc_t")
            nc.sync.dma_start(enc_t[:], enc[b].rearrange("(c p) h -> p c h", p=128))
            psum_c = pc_pool.tile([1, H], F32, name="psum_c")
            for c in range(SC):
                nc.tensor.matmul(
                    psum_c[0:1, :],
                    lhsT=attn_T[:, c : c + 1],
                    rhs=enc_t[:, c, :],
                    start=(c == 0),
                    stop=(c == SC - 1),
                )
            pe_warm(WARM_PER_B)
            cvec_row = spool.tile([1, H], F32, tag="cvecrow", name="cvec_row")
            nc.scalar.copy(cvec_row[:], psum_c[0:1, :])
            nc.sync.dma_start(cvec[b : b + 1, :], cvec_row[:])


_NC = None


def build_nc():
    global _NC
    if _NC is not None:
        return _NC
    nc = bacc.Bacc("TRN2", target_bir_lowering=False, debug=False)
    aps = {}
    for name, shape in [
        ("decT", [D, BL]),
        ("wdec", [D, U]),
        ("bdec", [128, UC]),
        ("wcov", [128, UC]),
    ]:
        aps[name] = nc.dram_tensor(name, shape, F32, kind="ExternalInput").ap()
    for name, shape in [
        ("cov", [BL, S]),
        ("enc", [BL, S, H]),
        ("encT", [BL, U, S]),
        ("watt", [128, UC]),
    ]:
        aps[name] = nc.dram_tensor(name, shape, BF16, kind="ExternalInput").ap()
    for name, shape in [("attn", [BL, S]), ("cvec", [BL, H])]:
        aps[name] = nc.dram_tensor(name, shape, F32, kind="ExternalOutput").ap()

    with tile.TileContext(nc) as tc:
        _build_body(tc, aps)
    nc.compile()
    _NC = nc
    return nc


def make_in_maps(enc_output, enc_attn, coverage_vector, dec_state, W_dec, b_dec,
                 W_cov, w_attn):
    enc_output = np.asarray(enc_output, dtype=np.float32)
    enc_attn = np.asarray(enc_attn, dtype=np.float32)
    coverage_vector = np.asarray(coverage_vector, dtype=np.float32).astype(
        ml_dtypes.bfloat16
    )
    dec_state = np.asarray(dec_state, dtype=np.float32)
    wdec = np.ascontiguousarray(np.asarray(W_dec, dtype=np.float32))
    bdec = np.ascontiguousarray(np.asarray(b_dec, dtype=np.float32).reshape(UC, 128).T)
    wcov = np.ascontiguousarray(
        np.asarray(W_cov, dtype=np.float32)[0].reshape(UC, 128).T
    )
    watt = np.ascontiguousarray(
        np.asarray(w_attn, dtype=np.float32)[:, 0].reshape(UC, 128).T
    ).astype(ml_dtypes.bfloat16)
    in_maps = []
    for i in range(NCORES):
        sl = slice(i * BL, (i + 1) * BL)
        in_maps.append({
            "encT": np.ascontiguousarray(
                enc_attn[sl].transpose(0, 2, 1)
            ).astype(ml_dtypes.bfloat16),
            "enc": np.ascontiguousarray(enc_output[sl]).astype(ml_dtypes.bfloat16),
            "cov": np.ascontiguousarray(coverage_vector[sl]),
            "decT": np.ascontiguousarray(dec_state[sl].T),
            "wdec": wdec,
            "bdec": bdec,
            "wcov": wcov,
            "watt": watt,
        })
    return in_maps


def kernel(enc_output, enc_attn, coverage_vector, dec_state, W_dec, b_dec,
           W_cov, w_attn, **run_kwargs):
    global LAST_RESULTS
    nc = build_nc()
    in_maps = make_in_maps(enc_output, enc_attn, coverage_vector, dec_state,
                           W_dec, b_dec, W_cov, w_attn)
    res = bass_utils.run_bass_kernel_spmd(
        nc, in_maps, core_ids=list(range(NCORES)), **run_kwargs
    )
    LAST_RESULTS = res
    attn = np.concatenate([r["attn"] for r in res.results], axis=0)
    cvec = np.concatenate([r["cvec"] for r in res.results], axis=0)
    return attn.reshape(B, S, 1), cvec
